# revision 5
# baseline (speedup 1.0000x reference)
"""Trainium2 Bass kernel for nn_EquivariantInteractionBlock (v3).

Changes vs v2 baseline:
- Fused w2 matmuls: per subtile A-mm (512 cols = p1|p2|p3), B-mm (64 = p4),
  G-mm (gate, 1 col): 24 mm/sup instead of 32.
- PSUM: pA [128,2,512] double-buffered per half-g (no PE<->drain ping-pong),
  pB [128,4,72] per g, hp per g, agg [128,505] double-buffered.
- Drains 3-way split: Act (p1), DVE (p2), Pool (p3); pB on Act.
- Path3: tree-add i-contraction (bf16 2x) instead of 1x reduce.
- Path4: one tree step (i8->i4), scatter carries (c,j,i4); P width 505
  -> single scatter matmul per subtile (8/sup instead of 16).
- Gate bias folded into G-mm via hg ones-row (wg2 [65,1], row64 = b_g2).
- Node pass C merged into one matmul (block-diagonal [112,40] weight).
- Host DMA layouts made round-contiguous (no strided rearrange descriptors).
"""

import math
from contextlib import ExitStack

import numpy as np

import concourse.bass as bass
import concourse.mybir as mybir
import concourse.tile as tile
from concourse.bass_utils import run_bass_kernel_spmd
from concourse.masks import make_identity

F32 = mybir.dt.float32
BF16 = mybir.dt.bfloat16
I32 = mybir.dt.int32
AF = mybir.ActivationFunctionType
OP = mybir.AluOpType

N = 50000
E = 400000
MUL0 = 16
MUL1 = 8
RBF = 8
HID = 64
CUTOFF = 5.0
EPS = 1e-8
INV3 = float(1.0 / np.sqrt(np.float32(3.0)))
APATH = float(1.0 / math.sqrt(MUL0 + MUL1))
NCORE = 8
SUB = 128          # edges per subtile
SPS = 8            # subtiles per supertile
SUPE = SUB * SPS   # 1024 edges per supertile
NPW = 128          # node window per supertile
RND = 4            # supertiles per DMA round
PAIR = 2           # supertiles per batched vector stage

# P (per-subtile product) column layout, 505 bf16 cols
C_P1 = 0      # 256: p1*g1 (j16,i16)
C_P2 = 256    # 128: p2*u2 (j16,i8)
C_P4 = 384    # 96: p4 half-contracted (c3,j8,i4)
C_EW = 480    # 1: edge weight
C_M13 = 481   # 24: m1 path3 (c3,j8), fully contracted per edge
PC = 505
# aggs (node-major, per sup) column layout (same as v2 node phase)
A_NRM = 0     # 1: norm (from ew channel)
A_M13 = 1     # 24: m1 path3 (c,j)
A_M0A = 25    # 16: m0 from path1
A_M0B = 41    # 16: m0 from path2
A_M14 = 57    # 24: m1 path4 (c,j)
AC = 81
SVS = 56


def _host_prep(x, edge_src, edge_dst, edge_sh, edge_rbf, edge_len,
               w_r1, b_r1, w_r2, b_r2, w_g1, b_g1, w_g2, b_g2,
               Wm_s, Wm_v, Wu_s, Wu_v, Ws_s, Ws_v, res_scale):
    import ml_dtypes
    BF = ml_dtypes.bfloat16
    order = np.argsort(edge_dst, kind="stable")
    src_s = edge_src[order]
    dst_s = edge_dst[order]
    sh_s = edge_sh[order]
    rbf_s = edge_rbf[order]
    len_s = edge_len[order]

    deg = np.bincount(edge_dst, minlength=N).astype(np.int64)
    cum = np.concatenate([[0], np.cumsum(deg)])

    bounds = [0]
    for k in range(1, NCORE):
        bounds.append(int(np.searchsorted(cum, k * E // NCORE)))
    bounds.append(N)

    cores = []
    for k in range(NCORE):
        n0, n1 = bounds[k], bounds[k + 1]
        sups = []
        nb = n0
        while nb < n1:
            nn = nb
            cnt = 0
            while nn < n1 and nn - nb < NPW and cnt + deg[nn] <= SUPE:
                cnt += int(deg[nn])
                nn += 1
            sups.append((nb, int(cum[nb]), cnt))
            nb = nn
        cores.append((n0, n1, sups))

    nsup = max(len(c[2]) for c in cores)
    nsup = -(-nsup // RND) * RND  # round up to multiple of RND
    nr = nsup // RND

    s0 = 1.0 / math.sqrt(MUL0)
    s1 = 1.0 / math.sqrt(MUL1)

    # shared weights
    w1p = np.zeros((9, 128), np.float32)
    w1p[:8, :64] = w_r1
    w1p[:8, 64:] = w_g1
    w1p[8, :64] = b_r1
    w1p[8, 64:] = b_g1

    # w2p: A block [.,0:512] = p1 (j16,i16) | p2 (j16,i8) | p3 (j8,i16);
    # B block [.,512:576] = p4 (j8,i8). (j outer, i inner everywhere)
    w2p = np.zeros((65, 576), np.float32)
    wsrc = np.concatenate([w_r2, b_r2[None, :]], axis=0)
    jj, ii = np.meshgrid(np.arange(16), np.arange(16), indexing="ij")
    w2p[:, (jj * 16 + ii).ravel()] = wsrc[:, (ii * 16 + jj).ravel()]
    jj, ii = np.meshgrid(np.arange(16), np.arange(8), indexing="ij")
    w2p[:, (256 + jj * 8 + ii).ravel()] = wsrc[:, (256 + ii * 16 + jj).ravel()]
    jj, ii = np.meshgrid(np.arange(8), np.arange(16), indexing="ij")
    w2p[:, (384 + jj * 16 + ii).ravel()] = wsrc[:, (384 + ii * 8 + jj).ravel()]
    jj, ii = np.meshgrid(np.arange(8), np.arange(8), indexing="ij")
    w2p[:, (512 + jj * 8 + ii).ravel()] = wsrc[:, (512 + ii * 8 + jj).ravel()]

    # gate weights with bias row (pairs with hg ones-row)
    wg2p = np.zeros((65, 1), np.float32)
    wg2p[:64, 0] = w_g2[:, 0]
    wg2p[64, 0] = float(b_g2[0]) + 1e-30

    # node-phase weights: one [AC, 49] matmul (col 0 norm, 1:25 sc, 25:49 vv)
    wmv = np.zeros((24, 24), np.float32)   # rows (c,j) -> cols (c',j')
    wuv = np.zeros((24, 24), np.float32)   # rows (c,j) -> cols (j',c')
    wsv = np.zeros((24, 24), np.float32)   # rows (j,c) -> cols (j',c')
    for c in range(3):
        for j in range(8):
            for j2 in range(8):
                wmv[c * 8 + j, c * 8 + j2] = Wm_v[j, j2] * s1
                wuv[c * 8 + j, j2 * 3 + c] = Wu_v[j, j2] * s1
                wsv[j * 3 + c, j2 * 3 + c] = Ws_v[j, j2] * s1
    wnode = np.zeros((AC, 49), np.float32)
    wnode[A_NRM, 0] = 1.0
    wnode[A_M0A:A_M0A + 16, 1:25] = Wm_s * s0
    wnode[A_M0B:A_M0B + 16, 1:25] = Wm_s * s0
    wnode[A_M13:A_M13 + 24, 25:49] = wmv
    wnode[A_M14:A_M14 + 24, 25:49] = wmv
    wus = (Wu_s * s0).astype(np.float32)
    wss = (Ws_s * s0 * 4.0).astype(np.float32)          # fold node inv_s = 4*rsqrt
    wsv = wsv * math.sqrt(8.0)                          # fold node inv_v
    # merged pass-C weight [112, 40]: rows 0:16 scal->Wu_s, 16:32 xns_s->Ws_s,
    # rows 64:88 vg->wuv, 88:112 xns_v->wsv; rows 32:64 zero.
    wnC = np.zeros((112, 40), np.float32)
    wnC[0:16, 0:16] = wus
    wnC[16:32, 0:16] = wss
    wnC[64:88, 16:40] = wuv
    wnC[88:112, 16:40] = wsv

    shared = dict(
        w1p=w1p.astype(BF), w2p=w2p.astype(BF),
        wg2=np.ascontiguousarray(wg2p).astype(BF),
        wnode=wnode.astype(BF),
        wnc=wnC.astype(BF),
    )

    in_maps = []
    metas = []
    for k in range(NCORE):
        n0, n1, sups = cores[k]
        ns = len(sups)
        idx = np.full((nsup, SUPE), -1, np.int64)
        base_arr = np.full((nsup,), n1, np.int64)
        span_arr = np.zeros((nsup,), np.int64)
        for si, (nb, es, cnt) in enumerate(sups):
            idx[si, :cnt] = np.arange(es, es + cnt)
            base_arr[si] = nb
            span_arr[si] = min(NPW, n1 - nb)
        mask = idx >= 0
        ic = np.clip(idx, 0, E - 1)

        feat = x[src_s[ic]]                                        # [nsup,SUPE,40]
        shp = np.where(mask[..., None], sh_s[ic], 0.0)
        lenp = np.where(mask & (len_s[ic] < CUTOFF), len_s[ic], 1.2 * CUTOFF)
        rbfp = np.where(mask[..., None], rbf_s[ic], 0.0)
        dstl = np.where(mask, dst_s[ic] - base_arr[:, None], -1)   # -1 -> zero sel row

        fs = np.concatenate([feat, shp, lenp[..., None]], axis=-1)  # [nsup,SUPE,45]
        # swizzle to [nr, 128, RND, SPS, 45] (round-contiguous per partition)
        fs = fs.reshape(nr, RND, SPS, SUB, 45).transpose(0, 3, 1, 2, 4)

        sel = np.zeros((nsup, SPS, SUB, NPW), np.float32)
        si_, sp_, su_ = np.meshgrid(np.arange(nsup), np.arange(SPS),
                                    np.arange(SUB), indexing="ij")
        d = dstl.reshape(nsup, SPS, SUB)
        valid = d >= 0
        sel[si_[valid], sp_[valid], su_[valid], d[valid]] = 1.0
        # [nsup,SPS,SUB,128] -> [nr, 128, RND, SPS, 128]
        sel = sel.reshape(nr, RND, SPS, SUB, NPW).transpose(0, 3, 1, 2, 4)

        rbft = np.concatenate(
            [rbfp.reshape(nsup, SUPE, 8).transpose(0, 2, 1),
             np.ones((nsup, 1, SUPE), np.float32)], axis=1)        # [nsup,9,1024]
        rbft = rbft.reshape(nr, RND, 9, SUPE).transpose(0, 2, 1, 3)  # [nr,9,RND,1024]

        nodes = np.clip(base_arr[:, None] + np.arange(NPW)[None, :], 0, N - 1)
        xop = x[nodes].transpose(1, 0, 2)                          # [128,nsup,40]

        m = dict(shared)
        m.update(fs=np.ascontiguousarray(fs).astype(BF),
                 sel=np.ascontiguousarray(sel).astype(BF),
                 rbft=np.ascontiguousarray(rbft).astype(BF),
                 xop=np.ascontiguousarray(xop).astype(np.float32))
        in_maps.append(m)
        metas.append((n0, n1, base_arr, span_arr, ns))

    return in_maps, metas, nsup, float(b_g2[0]), float(res_scale)


def build_program(nsup, bg2, res, stage=99):
    import concourse.bacc as bacc
    nc = bacc.Bacc("TRN2", target_bir_lowering=False, debug=False,
                   num_devices=NCORE)
    nr = nsup // RND

    fs_d = nc.dram_tensor("fs", [nr, 128, RND, SPS, 45], BF16, kind="ExternalInput")
    sel_d = nc.dram_tensor("sel", [nr, 128, RND, SPS, 128], BF16, kind="ExternalInput")
    rbft_d = nc.dram_tensor("rbft", [nr, 9, RND, SUPE], BF16, kind="ExternalInput")
    xop_d = nc.dram_tensor("xop", [128, nsup, 40], F32, kind="ExternalInput")
    w1p_d = nc.dram_tensor("w1p", [9, 128], BF16, kind="ExternalInput")
    w2p_d = nc.dram_tensor("w2p", [65, 576], BF16, kind="ExternalInput")
    wg2_d = nc.dram_tensor("wg2", [65, 1], BF16, kind="ExternalInput")
    wnode_d = nc.dram_tensor("wnode", [AC, 49], BF16, kind="ExternalInput")
    wnc_d = nc.dram_tensor("wnc", [112, 40], BF16, kind="ExternalInput")
    out_d = nc.dram_tensor("out", [128, nsup, 40], F32, kind="ExternalOutput")

    c_efs = 4.0 * APATH          # inv_s fold
    c_efv = math.sqrt(8.0) * APATH

    with tile.TileContext(nc) as tc:
        with (
            tc.tile_pool(name="const", bufs=1) as cp,
            tc.tile_pool(name="io", bufs=2) as iop,
            tc.tile_pool(name="mid", bufs=2) as mp,
            tc.tile_pool(name="pp", bufs=3) as ppp,
            tc.tile_pool(name="batch", bufs=1) as bp,
            tc.tile_pool(name="nd", bufs=6) as ndp,
        ):
            ps_stack = ExitStack()
            ps_pa = ps_stack.enter_context(
                tc.tile_pool(name="ps_pa", bufs=2, space="PSUM"))
            ps_pb = ps_stack.enter_context(
                tc.tile_pool(name="ps_pb", bufs=1, space="PSUM"))
            ps_h = ps_stack.enter_context(
                tc.tile_pool(name="ps_h", bufs=1, space="PSUM"))
            ps_a = ps_stack.enter_context(
                tc.tile_pool(name="ps_a", bufs=1, space="PSUM"))
            # ---- constants ----
            w1p = cp.tile([9, 128], BF16, tag="w1p")
            w2p = cp.tile([65, 576], BF16, tag="w2p")
            wg2 = cp.tile([65, 1], BF16, tag="wg2")
            wnode = cp.tile([AC, 49], BF16, tag="wnode")
            wnc = cp.tile([112, 40], BF16, tag="wnc")
            for t, d in [(w1p, w1p_d), (w2p, w2p_d), (wg2, wg2_d),
                         (wnode, wnode_d), (wnc, wnc_d)]:
                nc.sync.dma_start(out=t[:], in_=d[:])
            identf = cp.tile([128, 128], F32, tag="identf")
            identb = cp.tile([128, 128], BF16, tag="identb")
            make_identity(nc, identf[:])
            nc.scalar.copy(out=identb[:], in_=identf[:])
            cpi2 = cp.tile([128, 1], F32, tag="cpi2")
            nc.gpsimd.memset(cpi2[:], math.pi / 2)

            # persistent hidden tiles (row 64 of hm/hg = ones for biases)
            Ps = [cp.tile([128, SPS, PC], BF16, tag=f"P{i}", name=f"P{i}")
                  for i in range(2)]
            for i in range(2):
                nc.gpsimd.memset(Ps[i][:, :, C_EW], 1.0)
            hms = [cp.tile([65, 1024], BF16, tag=f"hm{i}", name=f"hm{i}")
                   for i in range(2)]
            hgs = [cp.tile([65, 1024], BF16, tag=f"hg{i}", name=f"hg{i}")
                   for i in range(2)]
            for i in range(2):
                nc.gpsimd.memset(hms[i][64:65, :], 1.0)
                nc.gpsimd.memset(hgs[i][64:65, :], 1.0)

            # whole-core batched tiles
            xob = bp.tile([128, nsup, 40], F32, tag="xob")
            nc.sync.dma_start(out=xob[:], in_=xop_d[:])
            aggsb = bp.tile([128, nsup, AC], BF16, tag="aggsb")
            scnb = bp.tile([128, nsup, 49], F32, tag="scnb")
            svxn = bp.tile([128, nsup, 112], F32, tag="svxn")
            nc.gpsimd.memset(svxn[:, :, 32:64], 0.0)
            finb = bp.tile([128, nsup, 40], F32, tag="finb")

            def fast_rsqrt(y, x, y2, eng=None):
                """y <- rsqrt(x) (1 Newton step). y holds x on entry; x is the
                preserved copy; y2 is scratch. All same-shape f32 SBUF APs."""
                eng = eng or nc.vector
                eng.tensor_scalar(out=y.bitcast(I32), in0=y.bitcast(I32),
                                  scalar1=1, scalar2=None,
                                  op0=OP.arith_shift_right)
                eng.tensor_scalar(out=y.bitcast(I32), in0=y.bitcast(I32),
                                  scalar1=-1, scalar2=0x5F3759DF,
                                  op0=OP.mult, op1=OP.add)
                eng.tensor_tensor(out=y2, in0=y, in1=y, op=OP.mult)
                eng.tensor_tensor(out=y2, in0=y2, in1=x, op=OP.mult)
                eng.tensor_scalar(out=y2, in0=y2, scalar1=-0.5, scalar2=1.5,
                                  op0=OP.mult, op1=OP.add)
                eng.tensor_tensor(out=y, in0=y, in1=y2, op=OP.mult)

            # scatter + node-drain for one sup (emitted one sup late)
            def scatter_sup(P, selw, s):
                agg = ps_a.tile([128, PC], F32, tag="agg")
                for t in range(SPS):
                    nc.tensor.matmul(out=agg[:], lhsT=selw[:, t, :],
                                     rhs=P[:, t, :],
                                     start=(t == 0), stop=(t == SPS - 1))
                # drain: norm+m13 copy, then i-reductions
                nc.scalar.copy(out=aggsb[:, s, 0:25], in_=agg[:, C_EW:PC])
                lp = nc.allow_low_precision("bf16 aggregates, 2e-2 tol")
                lp.__enter__()
                nc.vector.reduce_sum(
                    out=aggsb[:, s, A_M0A:A_M0A + 16],
                    in_=agg[:, 0:256].rearrange("p (j i) -> p j i", i=16),
                    axis=mybir.AxisListType.X)
                nc.vector.reduce_sum(
                    out=aggsb[:, s, A_M0B:A_M0B + 16],
                    in_=agg[:, 256:384].rearrange("p (j i) -> p j i", i=8),
                    axis=mybir.AxisListType.X)
                nc.vector.reduce_sum(
                    out=aggsb[:, s, A_M14:A_M14 + 24],
                    in_=agg[:, 384:480].rearrange("p (cj i) -> p cj i", i=4),
                    axis=mybir.AxisListType.X)
                lp.__exit__(None, None, None)

            pending = []  # (P, selb, sr, s) awaiting scatter

            # ---- main loop over rounds ----
            for r in range(nr):
                fsb = iop.tile([128, RND, SPS, 45], BF16, tag="fsb")
                selb = iop.tile([128, RND, SPS, 128], BF16, tag="selb")
                rbfb = iop.tile([9, RND, SUPE], BF16, tag="rbfb")
                nc.sync.dma_start(out=fsb[:], in_=fs_d[r])
                nc.sync.dma_start(out=selb[:], in_=sel_d[r])
                nc.sync.dma_start(out=rbfb[:], in_=rbft_d[r])

                feats = fsb[:, :, :, 0:40]
                sh0r = fsb[:, :, :, 40]
                sh1r = fsb[:, :, :, 41:44]
                lenr = fsb[:, :, :, 44]

                # batched per-round chains (rms via DVE bf16 square, 2x)
                sqb = mp.tile([128, RND, SPS, 40], BF16, tag="sqb")
                nc.vector.tensor_tensor(out=sqb[:], in0=feats, in1=feats,
                                        op=OP.mult)
                msb = mp.tile([128, RND, 2, SPS], F32, tag="msb")
                nc.vector.reduce_sum(out=msb[:, :, 0, :], in_=sqb[:, :, :, 0:16],
                                     axis=mybir.AxisListType.X)
                nc.vector.reduce_sum(out=msb[:, :, 1, :], in_=sqb[:, :, :, 16:40],
                                     axis=mybir.AxisListType.X)
                nc.vector.tensor_scalar_add(out=msb[:, :, 0, :], in0=msb[:, :, 0, :],
                                            scalar1=16 * EPS)
                nc.vector.tensor_scalar_add(out=msb[:, :, 1, :], in0=msb[:, :, 1, :],
                                            scalar1=8 * EPS)
                msb_x = mp.tile([128, RND, 2, SPS], F32, tag="msbx")
                msb_y2 = mp.tile([128, RND, 2, SPS], F32, tag="msby2")
                nc.gpsimd.tensor_copy(out=msb_x[:], in_=msb[:])
                fast_rsqrt(msb[:], msb_x[:], msb_y2[:])
                lcb = mp.tile([128, RND, SPS], F32, tag="lcb")
                nc.vector.tensor_scalar_min(out=lcb[:], in0=lenr, scalar1=CUTOFF)
                cosb = mp.tile([128, RND, SPS], F32, tag="cosb")
                nc.scalar.activation(out=cosb[:], in_=lcb[:], func=AF.Sin,
                                     scale=-math.pi / CUTOFF, bias=cpi2[:])
                # per-edge scalars (no ew): tsa = invs*c_efs*sh0, tsb = invs*c_efs,
                # tv = invv*c_efv*sh0 (path4), tv3 = invv*c_efv*INV3 (path2)
                invs = msb[:, :, 0, :]
                invv = msb[:, :, 1, :]
                tsa = mp.tile([128, RND, SPS], F32, tag="tsa")
                nc.vector.scalar_tensor_tensor(out=tsa[:], in0=invs,
                                               scalar=c_efs, in1=sh0r,
                                               op0=OP.mult, op1=OP.mult)
                tsb = mp.tile([128, RND, SPS], F32, tag="tsb")
                nc.vector.tensor_scalar_mul(out=tsb[:], in0=invs, scalar1=c_efs)
                tv = mp.tile([128, RND, SPS], F32, tag="tv")
                nc.vector.scalar_tensor_tensor(out=tv[:], in0=invv,
                                               scalar=c_efv, in1=sh0r,
                                               op0=OP.mult, op1=OP.mult)
                tv3 = mp.tile([128, RND, SPS], F32, tag="tv3")
                nc.vector.tensor_scalar_mul(out=tv3[:], in0=invv,
                                            scalar1=c_efv * INV3)
                # factor tile fct [128,RND,SPS,64]: 0:16 g1f, 16:32 f3f,
                # 32:40 u2f, 40:64 vtp (c,i)
                fct = mp.tile([128, RND, SPS, 64], BF16, tag="fct")
                nc.gpsimd.tensor_tensor(
                    out=fct[:, :, :, 0:16], in0=feats[:, :, :, 0:16],
                    in1=tsa[:, :, :, None].to_broadcast([128, RND, SPS, 16]),
                    op=OP.mult)
                nc.gpsimd.tensor_tensor(
                    out=fct[:, :, :, 16:32], in0=feats[:, :, :, 0:16],
                    in1=tsb[:, :, :, None].to_broadcast([128, RND, SPS, 16]),
                    op=OP.mult)
                vsh = mp.tile([128, RND, SPS, 8, 3], BF16, tag="vsh")
                nc.gpsimd.tensor_tensor(
                    out=vsh[:],
                    in0=feats[:, :, :, 16:40].rearrange(
                        "p r s (i c) -> p r s i c", c=3),
                    in1=sh1r[:, :, :, None, :].to_broadcast(
                        [128, RND, SPS, 8, 3]),
                    op=OP.mult)
                u2raw = mp.tile([128, RND, SPS, 8], F32, tag="u2raw")
                u2h = mp.tile([128, RND, SPS, 8], F32, tag="u2h")
                nc.vector.tensor_tensor(out=u2h[:], in0=vsh[:, :, :, :, 0],
                                        in1=vsh[:, :, :, :, 1], op=OP.add)
                nc.vector.tensor_tensor(out=u2raw[:], in0=u2h[:],
                                        in1=vsh[:, :, :, :, 2], op=OP.add)
                lp0 = nc.allow_low_precision("bf16 factors")
                lp0.__enter__()
                nc.vector.tensor_tensor(
                    out=fct[:, :, :, 32:40], in0=u2raw[:],
                    in1=tv3[:, :, :, None].to_broadcast([128, RND, SPS, 8]),
                    op=OP.mult)
                lp0.__exit__(None, None, None)
                nc.gpsimd.tensor_tensor(
                    out=fct[:, :, :, 40:64].rearrange(
                        "p r s (c i) -> p r s c i", c=3),
                    in0=feats[:, :, :, 16:40].rearrange(
                        "p r s (i c) -> p r s c i", c=3),
                    in1=tv[:, :, :, None, None].to_broadcast(
                        [128, RND, SPS, 3, 8]),
                    op=OP.mult)

                for sr in range(RND):
                    s = r * RND + sr
                    sp = s % 2
                    P = Ps[sp]
                    p123c = ppp.tile([128, SPS, 512], BF16, tag="p123c")
                    pBc = ppp.tile([128, SPS, 65], BF16, tag="pBc")
                    selw = ppp.tile([128, SPS, 128], BF16, tag="selw")
                    hp = ps_h.tile([128, 2, 512], F32, tag="hp")
                    for g in range(2):
                        nc.tensor.matmul(out=hp[:, g, :], lhsT=w1p[:],
                                         rhs=rbfb[:, sr, g * 512:(g + 1) * 512],
                                         start=True, stop=True)
                    nc.scalar.activation(
                        out=hms[sp][0:64, :].rearrange("p (g e) -> p g e", g=2),
                        in_=hp[0:64, :, :], func=AF.Silu)
                    nc.scalar.activation(
                        out=hgs[sp][0:64, :].rearrange("p (g e) -> p g e", g=2),
                        in_=hp[64:128, :, :], func=AF.Silu)
                    for g in range(2):
                        hm = hms[sp]
                        hg = hgs[sp]
                        g4 = slice(g * 4, g * 4 + 4)
                        pB = ps_pb.tile([128, 4, 72], F32, tag="pB")
                        for h in range(2):
                            pA = ps_pa.tile([128, 2, 512], F32, tag="pA")
                            for u in range(2):
                                tl = h * 2 + u
                                st = g * 4 + tl
                                lhs = hm[:, st * 128:(st + 1) * 128]
                                nc.tensor.matmul(out=pA[:, u, :], lhsT=lhs,
                                                 rhs=w2p[:, 0:512],
                                                 start=True, stop=True)
                                nc.tensor.matmul(out=pB[:, tl, 0:64], lhsT=lhs,
                                                 rhs=w2p[:, 512:576],
                                                 start=True, stop=True)
                                nc.tensor.matmul(out=pB[:, tl, 64:65],
                                                 lhsT=hg[:, st * 128:(st + 1) * 128],
                                                 rhs=wg2[:], start=True, stop=True)
                            t2 = slice(g * 4 + h * 2, g * 4 + h * 2 + 2)
                            nc.scalar.copy(out=p123c[:, t2, :], in_=pA[:])
                        nc.scalar.copy(out=pBc[:, g4, :], in_=pB[:, :, 0:65])
                    # gate chain, batched per sup (feeds only next-sup scatter)
                    sgz = mp.tile([128, SPS], F32, tag="sgz")
                    nc.scalar.activation(out=sgz[:], in_=pBc[:, :, 64],
                                         func=AF.Silu)
                    rz = mp.tile([128, SPS], F32, tag="rz")
                    nc.vector.reciprocal(out=rz[:], in_=pBc[:, :, 64])
                    nc.vector.tensor_tensor(out=rz[:], in0=sgz[:], in1=rz[:],
                                            op=OP.mult)
                    ewt = mp.tile([128, SPS], F32, tag="ewt")
                    nc.vector.scalar_tensor_tensor(
                        out=ewt[:], in0=cosb[:, sr, :], scalar=1.0,
                        in1=rz[:], op0=OP.add, op1=OP.mult)
                    nc.gpsimd.tensor_tensor(
                        out=selw[:], in0=selb[:, sr, :, :],
                        in1=ewt[:, :, None].to_broadcast([128, SPS, 128]),
                        op=OP.mult)
                    for g in range(2):
                        g4 = slice(g * 4, g * 4 + 4)
                        # ---- products for this g (4 subtiles, no ew) ----
                        fc = fct[:, sr, g4, :]
                        p4z = mp.tile([128, 4, 3, 8, 8], BF16, tag="p4z")
                        for c in range(3):
                            nc.vector.tensor_tensor(
                                out=p4z[:, :, c, :, :],
                                in0=pBc[:, g4, 0:64].rearrange(
                                    "p s (j i) -> p s j i", i=8),
                                in1=fct[:, sr, g4, None, 40 + c * 8:48 + c * 8]
                                .to_broadcast([128, 4, 8, 8]),
                                op=OP.mult)
                        nc.vector.tensor_tensor(
                            out=P[:, g4, C_P4:C_P4 + 96].rearrange(
                                "p s (cj i) -> p s cj i", i=4),
                            in0=p4z.rearrange(
                                "p s c j (h i) -> p s (c j) h i", h=2)[
                                :, :, :, 0, :],
                            in1=p4z.rearrange(
                                "p s c j (h i) -> p s (c j) h i", h=2)[
                                :, :, :, 1, :], op=OP.add)

                        nc.vector.tensor_tensor(
                            out=P[:, g4, C_P1:C_P1 + 256].rearrange(
                                "p s (j i) -> p s j i", i=16),
                            in0=p123c[:, g4, 0:256].rearrange(
                                "p s (j i) -> p s j i", i=16),
                            in1=fc[:, :, None, 0:16].to_broadcast(
                                [128, 4, 16, 16]),
                            op=OP.mult)
                        nc.vector.tensor_tensor(
                            out=P[:, g4, C_P2:C_P2 + 128].rearrange(
                                "p s (j i) -> p s j i", i=8),
                            in0=p123c[:, g4, 256:384].rearrange(
                                "p s (j i) -> p s j i", i=8),
                            in1=fc[:, :, None, 32:40].to_broadcast(
                                [128, 4, 16, 8]),
                            op=OP.mult)
                        z3 = mp.tile([128, 4, 8, 16], BF16, tag="z3")
                        nc.vector.tensor_tensor(
                            out=z3[:],
                            in0=p123c[:, g4, 384:512].rearrange(
                                "p s (j i) -> p s j i", i=16),
                            in1=fc[:, :, None, 16:32].to_broadcast(
                                [128, 4, 8, 16]),
                            op=OP.mult)
                        # tree-contract i16 -> 1 (bf16 2x adds)
                        t3a = mp.tile([128, 4, 8, 8], BF16, tag="t3a")
                        nc.vector.tensor_tensor(out=t3a[:], in0=z3[:, :, :, 0:8],
                                                in1=z3[:, :, :, 8:16], op=OP.add)
                        t3b = mp.tile([128, 4, 8, 4], BF16, tag="t3b")
                        nc.vector.tensor_tensor(out=t3b[:], in0=t3a[:, :, :, 0:4],
                                                in1=t3a[:, :, :, 4:8], op=OP.add)
                        t3c = mp.tile([128, 4, 8, 2], BF16, tag="t3c")
                        nc.vector.tensor_tensor(out=t3c[:], in0=t3b[:, :, :, 0:2],
                                                in1=t3b[:, :, :, 2:4], op=OP.add)
                        t3f = mp.tile([128, 4, 8], BF16, tag="t3f")
                        nc.vector.tensor_tensor(out=t3f[:], in0=t3c[:, :, :, 0],
                                                in1=t3c[:, :, :, 1], op=OP.add)
                        nc.gpsimd.tensor_tensor(
                            out=P[:, g4, C_M13:C_M13 + 24].rearrange(
                                "p s (c j) -> p s c j", j=8),
                            in0=t3f[:, :, None, :].to_broadcast([128, 4, 3, 8]),
                            in1=sh1r[:, sr, g4, :, None].to_broadcast(
                                [128, 4, 3, 8]),
                            op=OP.mult)
                    # scatter previous sup now (fills PE while products run)
                    if pending:
                        scatter_sup(*pending.pop())
                    pending.append((P, selw, s))

            if pending:
                scatter_sup(*pending.pop())

            if stage < 2:
                nc.scalar.copy(out=finb[:, :, 0:40],
                               in_=aggsb[:, :, 0:40])
                nc.sync.dma_start(out=out_d[:], in_=finb[:])

            # ---- node phase ----
            def node_phase():
                # pass A: per-sup transpose + combined msg matmul
                ctxA = tc.tile_pool(name="ps_nA", bufs=1, space="PSUM")
                ps_n1 = ctxA.__enter__()
                for s in range(nsup):
                    tp = ps_n1.tile([AC, 128], BF16, tag="tp", bufs=4)
                    nc.tensor.transpose(out=tp[:], in_=aggsb[:, s, :],
                                        identity=identb[:])
                    aggT = ndp.tile([AC, 128], BF16, tag="aggT")
                    if s % 2 == 0:
                        nc.vector.tensor_copy(out=aggT[:], in_=tp[:])
                    else:
                        nc.scalar.copy(out=aggT[:], in_=tp[:])
                    scv = ps_n1.tile([128, 49], F32, tag="scv", bufs=4)
                    nc.tensor.matmul(out=scv[:], lhsT=aggT[:], rhs=wnode[:],
                                     start=True, stop=True)
                    if s % 2 == 0:
                        nc.scalar.copy(out=scnb[:, s, :], in_=scv[:])
                    else:
                        nc.vector.tensor_copy(out=scnb[:, s, :], in_=scv[:])

                ctxA.__exit__(None, None, None)
                # pass B: batched node elementwise
                invn = bp.tile([128, nsup], F32, tag="invn")
                nc.vector.tensor_scalar_max(out=invn[:], in0=scnb[:, :, 0],
                                            scalar1=EPS)
                nc.vector.reciprocal(out=invn[:], in_=invn[:])
                scn = scnb[:, :, 1:25]
                nc.gpsimd.tensor_tensor(
                    out=scn, in0=scn,
                    in1=invn[:, :, None].to_broadcast([128, nsup, 24]), op=OP.mult)
                nc.scalar.activation(out=svxn[:, :, 0:16], in_=scnb[:, :, 1:17],
                                     func=AF.Silu)
                zg = bp.tile([128, nsup, 8], F32, tag="zg")
                nc.vector.tensor_scalar_add(out=zg[:], in0=scnb[:, :, 17:25],
                                            scalar1=1e-30)
                sgg = bp.tile([128, nsup, 8], F32, tag="sgg")
                nc.scalar.activation(out=sgg[:], in_=zg[:], func=AF.Silu)
                rzg = bp.tile([128, nsup, 8], F32, tag="rzg")
                nc.vector.reciprocal(out=rzg[:], in_=zg[:])
                gts = bp.tile([128, nsup, 8], F32, tag="gts")
                nc.gpsimd.tensor_tensor(out=gts[:], in0=sgg[:], in1=rzg[:],
                                        op=OP.mult)
                vvn = scnb[:, :, 25:49]
                nc.gpsimd.tensor_tensor(
                    out=vvn, in0=vvn,
                    in1=invn[:, :, None].to_broadcast([128, nsup, 24]), op=OP.mult)
                nc.vector.tensor_tensor(
                    out=svxn[:, :, 64:88].rearrange("p s (c j) -> p s c j", j=8),
                    in0=vvn.rearrange("p s (c j) -> p s c j", j=8),
                    in1=gts[:, :, None, :].to_broadcast([128, nsup, 3, 8]),
                    op=OP.mult)
                # x own-node rms
                xsq = bp.tile([128, nsup, 40], BF16, tag="xsq")
                nc.scalar.activation(out=xsq[:], in_=xob[:], func=AF.Square)
                xms = bp.tile([128, nsup, 2], F32, tag="xms")
                nc.vector.reduce_sum(out=xms[:, :, 0], in_=xsq[:, :, 0:16],
                                     axis=mybir.AxisListType.X)
                nc.vector.reduce_sum(out=xms[:, :, 1], in_=xsq[:, :, 16:40],
                                     axis=mybir.AxisListType.X)
                nc.vector.tensor_scalar_add(out=xms[:, :, 0], in0=xms[:, :, 0],
                                            scalar1=16 * EPS)
                nc.vector.tensor_scalar_add(out=xms[:, :, 1], in0=xms[:, :, 1],
                                            scalar1=8 * EPS)
                xmsx = bp.tile([128, nsup, 2], F32, tag="xmsx")
                y2t = bp.tile([128, nsup, 2], F32, tag="y2t")
                nc.scalar.copy(out=xmsx[:], in_=xms[:])
                fast_rsqrt(xms[:], xmsx[:], y2t[:])
                nc.gpsimd.tensor_tensor(
                    out=svxn[:, :, 16:32], in0=xob[:, :, 0:16],
                    in1=xms[:, :, 0:1].to_broadcast([128, nsup, 16]), op=OP.mult)
                nc.gpsimd.tensor_tensor(
                    out=svxn[:, :, 88:112], in0=xob[:, :, 16:40],
                    in1=xms[:, :, 1:2].to_broadcast([128, nsup, 24]), op=OP.mult)

                if stage < 3:
                    nc.scalar.copy(out=finb[:, :, 0:40],
                                   in_=svxn[:, :, 0:40])
                    nc.sync.dma_start(out=out_d[:], in_=finb[:])
                    return

                # pass C: per-sup transpose + single update matmul
                ctxC = tc.tile_pool(name="ps_nC", bufs=1, space="PSUM")
                ps_n3 = ctxC.__enter__()
                for s in range(nsup):
                    tps = ps_n3.tile([112, 128], F32, tag="tps", bufs=4)
                    nc.tensor.transpose(out=tps[:], in_=svxn[:, s, :],
                                        identity=identf[:])
                    svxT = ndp.tile([112, 128], BF16, tag="svxT")
                    if s % 2 == 0:
                        nc.vector.tensor_copy(out=svxT[:], in_=tps[:])
                    else:
                        nc.scalar.copy(out=svxT[:], in_=tps[:])
                    outp = ps_n3.tile([128, 40], F32, tag="outp", bufs=4)
                    nc.tensor.matmul(out=outp[:], lhsT=svxT[:],
                                     rhs=wnc[:], start=True, stop=True)
                    if s % 2 == 0:
                        nc.scalar.copy(out=finb[:, s, :], in_=outp[:])
                    else:
                        nc.vector.tensor_copy(out=finb[:, s, :], in_=outp[:])

                ctxC.__exit__(None, None, None)
                if stage < 4:
                    nc.sync.dma_start(out=out_d[:], in_=finb[:])
                    return
                # pass D: residual + output
                fino = scnb[:, :, 0:40]  # scnb is dead after pass B; reuse
                nc.vector.scalar_tensor_tensor(out=fino, in0=finb[:],
                                               scalar=res, in1=xob[:],
                                               op0=OP.mult, op1=OP.add)
                nc.sync.dma_start(out=out_d[:], in_=fino)

            ps_stack.close()
            if stage >= 2:
                node_phase()

    nc.compile()
    return nc


_CACHE = {}


def kernel(**inputs):
    in_maps, metas, nsup, bg2, res = _host_prep(**inputs)
    key = (nsup, bg2, res)
    if key not in _CACHE:
        _CACHE[key] = build_program(nsup, bg2, res)
    nc = _CACHE[key]
    r = run_bass_kernel_spmd(nc, in_maps, list(range(NCORE)))
    out = np.zeros((N, 40), np.float32)
    for k in range(NCORE):
        n0, n1, base_arr, span_arr, ns = metas[k]
        ob = r.results[k]["out"]  # [128, nsup, 40]
        for si in range(ns):
            sp = int(span_arr[si])
            if sp > 0:
                b = int(base_arr[si])
                out[b:b + sp] = ob[:sp, si]
    return out
